# revision 5
# baseline (speedup 1.0000x reference)
"""TRN2 raw-bass Mixtral HQQ MLP: split-precision up-projection.

Up-projection contracts hid in two parts: the first 2*KP8 hid-tiles as
fp8-e4m3 DoubleRow pairs (halving those matmuls), the rest bf16.
Down-projection is fp8 DoubleRow (as before). u is scaled 2^-15 on the
ScalarE copy (PSUM->SBUF) so no weight pre-scaling is needed.
"""

import os
import sys

for _p in ("/opt/trn_rl_repo", "/root/.axon_site/_ro/trn_rl_repo"):
    if os.path.isdir(_p) and _p not in sys.path:
        sys.path.insert(0, _p)

import ml_dtypes
import numpy as np

import concourse.bacc as bacc
import concourse.mybir as mybir
import concourse.bass as bass
from concourse.bass_utils import run_bass_kernel_spmd

BF16 = ml_dtypes.bfloat16
FP8 = ml_dtypes.float8_e4m3

N_CORES = 8
TOK = 4096
HID = 4096
INT = 14336
GS = 64

INT_SH = INT // N_CORES          # 1792
TB = 2048
NB = TOK // TB                   # 2
I_TILES = INT_SH // 128          # 14
H_TILES = HID // 128             # 32
KP8 = 8                          # fp8 hid k-PAIRS (fraction = KP8/16)
H_BF = H_TILES - 2 * KP8         # bf16 hid tiles
TBS = TB // 512                  # 4
HQS = 4
USCALE = 2.0 ** -15

_CACHE = {}


def _build_nc(repeats=1, sigmoid_for_sim=False):
    key = ("nc", repeats, sigmoid_for_sim)
    if key in _CACHE:
        return _CACHE[key]

    nc = bacc.Bacc("TRN2", target_bir_lowering=False, debug=False)
    bf = mybir.dt.bfloat16
    f8 = mybir.dt.float8e4
    f32 = mybir.dt.float32
    DR = mybir.MatmulPerfMode.DoubleRow
    Silu = (mybir.ActivationFunctionType.Sigmoid if sigmoid_for_sim
            else mybir.ActivationFunctionType.Silu)
    Copy = mybir.ActivationFunctionType.Copy

    x8_d = nc.dram_tensor("x8t", [NB, 128, KP8, 2, TB], f8, kind="ExternalInput")
    xb_d = nc.dram_tensor("xbt", [NB, 128, H_BF, TB], bf, kind="ExternalInput")
    w1f_d = nc.dram_tensor("w1f", [I_TILES, 128, KP8, 2, 128], f8,
                           kind="ExternalInput")
    w1b_d = nc.dram_tensor("w1b", [I_TILES, 128, H_BF, 128], bf,
                           kind="ExternalInput")
    w3f_d = nc.dram_tensor("w3f", [I_TILES, 128, KP8, 2, 128], f8,
                           kind="ExternalInput")
    w3b_d = nc.dram_tensor("w3b", [I_TILES, 128, H_BF, 128], bf,
                           kind="ExternalInput")
    w2_d = nc.dram_tensor("w2t", [2, 128, I_TILES // 2, 2, TB], f8,
                          kind="ExternalInput")
    out_d = nc.dram_tensor("out", [TOK, HID], bf, kind="ExternalOutput")

    x8_sb = nc.alloc_sbuf_tensor("x8_sb", [128, KP8, 2, TB], f8)
    xb_sb = nc.alloc_sbuf_tensor("xb_sb", [128, H_BF, TB], bf)
    w1f_sb = nc.alloc_sbuf_tensor("w1f_sb", [128, KP8, 2, 128], f8)
    w1b_sb = nc.alloc_sbuf_tensor("w1b_sb", [128, H_BF, 128], bf)
    w3f_sb = nc.alloc_sbuf_tensor("w3f_sb", [128, KP8, 2, 128], f8)
    w3b_sb = nc.alloc_sbuf_tensor("w3b_sb", [128, H_BF, 128], bf)
    h_sb = nc.alloc_sbuf_tensor("h_sb", [128, I_TILES, TB], f8)
    w2_sb = nc.alloc_sbuf_tensor("w2_sb", [128, I_TILES // 2, 2, TB], f8)
    sil_sb = nc.alloc_sbuf_tensor("sil_sb", [128, 4, 512], bf)
    us_sb = nc.alloc_sbuf_tensor("us_sb", [128, 4, 512], bf)
    o_sb = nc.alloc_sbuf_tensor("o_sb", [128, 2, 1024], bf)
    ps = [nc.alloc_psum_tensor(f"ps{i}", [128, 512], f32) for i in range(8)]

    s_x = nc.alloc_semaphore("s_x")
    s_w13 = nc.alloc_semaphore("s_w13")
    s_w1f = nc.alloc_semaphore("s_w1f")
    s_w3f = nc.alloc_semaphore("s_w3f")
    s_w2 = nc.alloc_semaphore("s_w2")
    s_sil = nc.alloc_semaphore("s_sil")    # scalar: 1 per (it, tb) ucopy
    s_mul = nc.alloc_semaphore("s_mul")
    s_pedn = nc.alloc_semaphore("s_pedn")
    s_ordy = nc.alloc_semaphore("s_ordy")
    s_ofree = [nc.alloc_semaphore("s_ofree0"), nc.alloc_semaphore("s_ofree1")]

    X_DMAS = 3                     # 1x x8 + 2x xb per block
    GRPS_PER_BLK = 2 * 16
    MULS_PER_BLK = I_TILES * TBS

    def blocks():
        for rep in range(repeats):
            for b in range(NB):
                yield rep * NB + b, b

    with nc.Block() as block:

        @block.sync
        def _(sync: bass.BassEngine):
            for bi, b in blocks():
                if bi >= 1:
                    sync.wait_ge(s_w3f, I_TILES * bi)
                sync.dma_start(x8_sb[:], x8_d[b]).then_inc(s_x, 16)
                hb2 = H_BF // 2
                sync.dma_start(xb_sb[:, 0:hb2, :],
                               xb_d[b, :, 0:hb2, :]).then_inc(s_x, 16)
                sync.dma_start(xb_sb[:, hb2:H_BF, :],
                               xb_d[b, :, hb2:H_BF, :]).then_inc(s_x, 16)
                for it in range(I_TILES):
                    itg = bi * I_TILES + it
                    if itg >= 1:
                        sync.wait_ge(s_w1f, itg)
                    sync.dma_start(w1f_sb[:], w1f_d[it]).then_inc(s_w13, 16)
                    sync.dma_start(w1b_sb[:], w1b_d[it]).then_inc(s_w13, 16)
                    if itg >= 1:
                        sync.wait_ge(s_w3f, itg)
                    sync.dma_start(w3f_sb[:], w3f_d[it]).then_inc(s_w13, 16)
                    sync.dma_start(w3b_sb[:], w3b_d[it]).then_inc(s_w13, 16)
                for hh in range(2):
                    prev_groups = bi * GRPS_PER_BLK + hh * 16
                    if prev_groups > 0:
                        sync.wait_ge(s_pedn, prev_groups)
                    sync.dma_start(w2_sb[:], w2_d[hh]).then_inc(s_w2, 16)

        @block.tensor
        def _(tensor: bass.BassEngine):
            grp = 0
            for bi, b in blocks():
                # ---------------- UP ----------------
                tensor.wait_ge(s_x, 16 * X_DMAS * (bi + 1))
                if bi >= 1:
                    tensor.wait_ge(s_ordy, 2 * bi * GRPS_PER_BLK)
                for it in range(I_TILES):
                    itg = bi * I_TILES + it
                    tensor.wait_ge(s_w13, 64 * (itg + 1))
                    if itg >= 1:
                        # scalar read all g/u banks of prev it (ucopy implies)
                        tensor.wait_ge(s_sil, TBS * itg)
                    for wf, wb, bank0, sfree in ((w1f_sb, w1b_sb, 0, s_w1f),
                                                 (w3f_sb, w3b_sb, 4, s_w3f)):
                        last = None
                        for kp in range(KP8):
                            wp = wf[:, kp, :, :]
                            tensor.ldweights(wp, perf_mode=DR)
                            for tb in range(TBS):
                                last = tensor.matmul(
                                    ps[bank0 + tb][:], wp,
                                    x8_sb[:, kp, :, tb * 512:(tb + 1) * 512],
                                    start=(kp == 0), stop=False, perf_mode=DR)
                                last.ins.ldweights = False
                        for ht in range(H_BF):
                            w = wb[:, ht, :]
                            tensor.ldweights(w)
                            for tb in range(TBS):
                                last = tensor.matmul(
                                    ps[bank0 + tb][:], w,
                                    xb_sb[:, ht, tb * 512:(tb + 1) * 512],
                                    start=False, stop=(ht == H_BF - 1))
                                last.ins.ldweights = False
                        last.then_inc(sfree, 1)
                # ---------------- DOWN ----------------
                tensor.wait_ge(s_mul, MULS_PER_BLK * (bi + 1))
                for hh in range(2):
                    tensor.wait_ge(s_w2, 16 * (2 * bi + hh + 1))
                    for tt in range(16):
                        if grp >= 2:
                            tensor.wait_ge(s_ordy, 2 * (grp - 1))
                        bs = (tt % 2) * 4
                        last = None
                        for itp in range(I_TILES // 2):
                            h_t = h_sb[:, 2 * itp:2 * itp + 2,
                                       tt * 128:(tt + 1) * 128]
                            tensor.ldweights(h_t, perf_mode=DR)
                            for hq in range(HQS):
                                last = tensor.matmul(
                                    ps[bs + hq][:], h_t,
                                    w2_sb[:, itp, :, hq * 512:(hq + 1) * 512],
                                    start=(itp == 0),
                                    stop=(itp == I_TILES // 2 - 1),
                                    perf_mode=DR)
                                last.ins.ldweights = False
                        last.then_inc(s_pedn, 1)
                        grp += 1

        @block.scalar
        def _(scalar: bass.BassEngine):
            for bi, b in blocks():
                for it in range(I_TILES):
                    itg = bi * I_TILES + it
                    scalar.wait_ge(s_w1f, itg + 1)
                    for tb in range(TBS):
                        j = itg * TBS + tb
                        if j >= 4:
                            scalar.wait_ge(s_mul, j - 3)
                        scalar.activation(sil_sb[:, j % 4, :], ps[tb][:], Silu)
                    scalar.wait_ge(s_w3f, itg + 1)
                    for tb in range(TBS):
                        j = itg * TBS + tb
                        scalar.activation(
                            us_sb[:, j % 4, :], ps[4 + tb][:], Copy,
                            0.0, USCALE,
                        ).then_inc(s_sil, 1)

        @block.vector
        def _(vector: bass.BassEngine):
            dma_i = 0
            for bi, b in blocks():
                for it in range(I_TILES):
                    itg = bi * I_TILES + it
                    for tb in range(TBS):
                        j = itg * TBS + tb
                        vector.wait_ge(s_sil, j + 1)
                        vector.tensor_mul(
                            h_sb[:, it, tb * 512:(tb + 1) * 512],
                            sil_sb[:, j % 4, :], us_sb[:, j % 4, :],
                        ).then_inc(s_mul, 1)
                for hh in range(2):
                    for tt in range(16):
                        g = bi * GRPS_PER_BLK + hh * 16 + tt
                        vector.wait_ge(s_pedn, g + 1)
                        bs = (tt % 2) * 4
                        last = None
                        for c in range(2):
                            if dma_i >= 2:
                                vector.wait_ge(s_ofree[dma_i % 2],
                                               16 * (dma_i // 2))
                            for k in range(2):
                                last = vector.tensor_copy(
                                    o_sb[:, dma_i % 2, k * 512:(k + 1) * 512],
                                    ps[bs + c * 2 + k][:])
                            last.then_inc(s_ordy, 1)
                            dma_i += 1

        @block.gpsimd
        def _(gpsimd: bass.BassGpSimd):
            dma_i = 0
            for bi, b in blocks():
                for hh in range(2):
                    for tt in range(16):
                        for c in range(2):
                            gpsimd.wait_ge(s_ordy, dma_i + 1)
                            rows = slice(b * TB + tt * 128,
                                         b * TB + (tt + 1) * 128)
                            col0 = hh * 2048 + c * 1024
                            gpsimd.dma_start(
                                out_d[rows, col0:col0 + 1024],
                                o_sb[:, dma_i % 2, :],
                            ).then_inc(s_ofree[dma_i % 2], 16)
                            dma_i += 1
            gpsimd.wait_ge(s_ofree[0], 16 * (dma_i // 2))
            gpsimd.wait_ge(s_ofree[1], 16 * (dma_i // 2))

    nc.compile()
    _CACHE[key] = nc
    return nc


def _dequant(q, s, z):
    out, inp = q.shape
    g = inp // GS
    qf = np.asarray(q, np.float32).reshape(out, g, GS)
    w = (qf - np.asarray(z, np.float32)[:, :, None]) * \
        np.asarray(s, np.float32)[:, :, None]
    return w.reshape(out, inp)


def _prep_in_maps(hidden_states, w1_q, w1_scale, w1_zero, w3_q, w3_scale,
                  w3_zero, w2_q, w2_scale, w2_zero):
    x = np.asarray(hidden_states, np.float32)
    n8 = 2 * KP8 * 128                   # fp8 hid columns
    # x8t[b, p, kp, j, t] = x[b*TB+t, (2kp+j)*128+p]
    x8t = np.ascontiguousarray(
        np.clip(x[:, :n8], -240, 240).astype(FP8)
        .reshape(NB, TB, KP8, 2, 128).transpose(0, 4, 2, 3, 1))
    # xbt[b, p, a, t] = x[b*TB+t, n8 + a*128+p]
    xbt = np.ascontiguousarray(
        x[:, n8:].astype(BF16).reshape(NB, TB, H_BF, 128).transpose(0, 3, 2, 1))

    def up_shard(q, s, z, c):
        rows = slice(c * INT_SH, (c + 1) * INT_SH)
        wd = _dequant(q[rows], s[rows], z[rows])
        wf = np.ascontiguousarray(
            np.clip(wd[:, :n8], -240, 240).astype(FP8)
            .reshape(I_TILES, 128, KP8, 2, 128).transpose(0, 4, 2, 3, 1))
        wb = np.ascontiguousarray(
            wd[:, n8:].astype(BF16)
            .reshape(I_TILES, 128, H_BF, 128).transpose(0, 3, 2, 1))
        return wf, wb

    def down_shard(q, s, z, c):
        cols = slice(c * INT_SH, (c + 1) * INT_SH)
        gsl = slice(c * (INT_SH // GS), (c + 1) * (INT_SH // GS))
        wd = _dequant(np.ascontiguousarray(q[:, cols]), s[:, gsl], z[:, gsl])
        wd8 = np.clip(wd, -240, 240).astype(FP8)
        return np.ascontiguousarray(
            wd8.reshape(2, TB, I_TILES // 2, 2, 128).transpose(0, 4, 2, 3, 1))

    in_maps = []
    for c in range(N_CORES):
        w1f, w1b = up_shard(w1_q, w1_scale, w1_zero, c)
        w3f, w3b = up_shard(w3_q, w3_scale, w3_zero, c)
        in_maps.append({
            "x8t": x8t, "xbt": xbt,
            "w1f": w1f, "w1b": w1b, "w3f": w3f, "w3b": w3b,
            "w2t": down_shard(w2_q, w2_scale, w2_zero, c),
        })
    return in_maps


def kernel(**inputs):
    nc = _build_nc()
    in_maps = _prep_in_maps(**inputs)
    res = run_bass_kernel_spmd(nc, in_maps, core_ids=list(range(N_CORES)))
    out = np.zeros((TOK, HID), np.float32)
    for c in range(N_CORES):
        out += res.results[c]["out"].astype(np.float32)
    return (out * np.float32(1.0 / USCALE)).astype(np.float32, copy=False)
